# revision 2
# baseline (speedup 1.0000x reference)
"""Trainium2 Bass kernel: dense attention with key-padding mask (ColoAttention).

Math (per batch b, head h):
    scores = (Q @ K^T) / sqrt(D); masked keys -> -inf; softmax over keys;
    out = probs @ V; rows at masked query positions zeroed.

v2 design notes (over the previous baseline):
  - Query widths are len-exact (not padded to nkc*128): sub-blocks are
    [1024, len-1024]; queries beyond len are never computed (host zeros).
  - exp is split across TWO engines.  ScalarE runs the exact LUT exp.  The
    DVE runs a Schraudolph exp2 for ~1/3 of the interior key-chunks: one
    stock tensor_scalar computes bits16 = rint((y+127+c)*128) with
    y = score/sqrt(D)*log2(e), written as int16 into the bf16 prob tile --
    those 16 bits ARE the bf16 pattern of ~2^y (rms log err ~1.8%, applied
    to ~30% of keys -> ~1% output error vs the 2e-2 gate).  The last chunk
    of every slot stays on ScalarE so masked keys contribute exactly
    exp(0)=1 (host subtracts the masked-key count from the row sums).
  - Narrow sub-blocks (w<=512) pair two key-chunks side-by-side in one
    PSUM score tile so one activation call covers both (halves the ~352
    cycle per-ACTIVATE fixed cost).
  - Row sums: prob chunks are pairwise tree-accumulated (bf16) with the
    adds alternating DVE / GpSimd, then one ones-vector matmul per
    sub-block reduces the 128 partitions exactly in PSUM.  Sums + O'
    drains are split between ScalarE copies and DVE copies; O' leaves via
    a single gpsimd DMA per sub-block.
  - PSUM: scores [128,1024]f32 x3 rotation (6 banks, sums matmuls ride the
    rotation with dummy allocations keeping parity) + O' accum
    [128,1024]f32 x1 (2 banks).
"""

import numpy as np
import ml_dtypes
from contextlib import ExitStack

import concourse.bass as bass
import concourse.mybir as mybir
import concourse.tile as tile
from concourse import bacc
from concourse.bass_utils import run_bass_kernel_spmd

B, S, H, D = 4, 2048, 16, 128
N_CORES = 8
P = 128
SCALE = 1.0 / float(np.sqrt(np.float64(D)).astype(np.float32))
LOG2E = float(np.log2(np.e))
SCHRAU_C = -0.057
# Schraudolph: bits16 = rint(score*(128*SCALE*LOG2E) + 128*(127+c))
SCHRAU_S1 = 128.0 * SCALE * LOG2E
SCHRAU_S2 = 128.0 * (127.0 + SCHRAU_C)
DVE_MOD = 3  # route group to DVE exp when g % DVE_MOD == 1 (and no last chunk)


def _subs_of(length: int):
    """[1024, len-1024] query sub-blocks (len-exact, no padding)."""
    if length <= 1024:
        return [(0, length)]
    return [(0, 1024), (1024, length - 1024)]


def _bank_windows(lo: int, hi: int):
    """Split [lo, hi) at 512-col PSUM bank boundaries."""
    res = []
    x = lo
    while x < hi:
        nxt = min(hi, (x // 512 + 1) * 512)
        res.append((x, nxt - x))
        x = nxt
    return res


def _groups_of(nkc: int, w: int):
    """Chunk groups for one sub-block: pairs when w<=512 (one activation
    covers both), singles otherwise."""
    if w <= 512:
        gs = [tuple(range(k, min(k + 2, nkc))) for k in range(0, nkc, 2)]
    else:
        gs = [(k,) for k in range(nkc)]
    return gs


def build_program(sched) -> bacc.Bacc:
    """sched: tuple of (nkc, length) per slot, identical on every core."""
    f32 = mybir.dt.float32
    bf16 = mybir.dt.bfloat16
    i16 = mybir.dt.int16
    Exp = mybir.ActivationFunctionType.Exp
    mult = mybir.AluOpType.mult
    add_op = mybir.AluOpType.add

    nc = bacc.Bacc("TRN2", target_bir_lowering=False, debug=False)
    q_d = nc.dram_tensor("q", [8, P, S], bf16, kind="ExternalInput").ap()
    k_d = nc.dram_tensor("k", [8, P, S], bf16, kind="ExternalInput").ap()
    v_d = nc.dram_tensor("v", [8, S, P], bf16, kind="ExternalInput").ap()
    out_d = nc.dram_tensor("out", [8, P, S], f32, kind="ExternalOutput").ap()
    sums_d = nc.dram_tensor("sums_out", [8, S], f32, kind="ExternalOutput").ap()

    # steps: one entry per chunk-group:
    # (slot, sub_off, sub_w, chunks, has_last_chunk, is_last_group)
    steps = []
    for s, (nkc, length) in enumerate(sched):
        for (off, w) in _subs_of(length):
            gs = _groups_of(nkc, w)
            for gi, chunks in enumerate(gs):
                steps.append((s, off, w, chunks, gi,
                              (nkc - 1) in chunks, gi == len(gs) - 1))

    with tile.TileContext(nc) as tc:
        with ExitStack() as ctx:
            consts = ctx.enter_context(tc.tile_pool(name="consts", bufs=1))
            qkp = ctx.enter_context(tc.tile_pool(name="qkp", bufs=2))
            ptp = ctx.enter_context(tc.tile_pool(name="ptp", bufs=4))
            treep = ctx.enter_context(tc.tile_pool(name="treep", bufs=8))
            otp = ctx.enter_context(tc.tile_pool(name="otp", bufs=4))
            smp = ctx.enter_context(tc.tile_pool(name="smp", bufs=2))
            sps = ctx.enter_context(tc.tile_pool(name="sps", bufs=3, space="PSUM"))
            ops = ctx.enter_context(tc.tile_pool(name="ops", bufs=1, space="PSUM"))

            ones_b = consts.tile([P, 1], bf16, tag="ones")
            nc.gpsimd.memset(ones_b[:], 1.0)

            staged = {}

            def stage(s):
                if s in staged or s >= len(sched):
                    return
                nkc, length = sched[s]
                kw = nkc * P
                q_sb = qkp.tile([P, length], bf16, tag="q", name=f"q_{s}")
                k_sb = qkp.tile([P, kw], bf16, tag="k", name=f"k_{s}")
                v_sb = qkp.tile([P, nkc, P], bf16, tag="v", name=f"v_{s}")
                # first compute needs k/v chunk 0 and the first q window;
                # the rest arrives as one large transfer each
                nc.sync.dma_start(k_sb[:, 0:P], k_d[s, :, 0:P])
                nc.sync.dma_start(q_sb[:, 0:512], q_d[s, :, 0:512])
                nc.sync.dma_start(v_sb[:, 0, :], v_d[s, 0:P, :])
                if length > 512:
                    nc.sync.dma_start(q_sb[:, 512:length], q_d[s, :, 512:length])
                if nkc > 1:
                    nc.sync.dma_start(k_sb[:, P:kw], k_d[s, :, P:kw])
                    nc.sync.dma_start(
                        v_sb[:, 1:nkc, :],
                        v_d[s, P:kw, :].rearrange("(t r) d -> r t d", r=P))
                staged[s] = (q_sb, k_sb, v_sb)

            stage(0)
            stage(1)

            def emit_qk(i):
                s, off, w, chunks, gi, _, _ = steps[i]
                stage(s + 1)
                q_sb, k_sb, _ = staged[s]
                s_ps = sps.tile([P, 1024], f32, tag="s", name=f"s_{i}")
                for j, kc in enumerate(chunks):
                    for (ws, ww) in _bank_windows(j * w, (j + 1) * w):
                        qs = off + ws - j * w
                        nc.tensor.matmul(
                            s_ps[:, ws:ws + ww],
                            lhsT=k_sb[:, kc * P:(kc + 1) * P],
                            rhs=q_sb[:, qs:qs + ww],
                            start=True, stop=True)
                return s_ps

            # binary-counter tree accumulation of prob chunks (per sub-block)
            levels = [None] * 6
            eng_ctr = [0]

            def tree_add(dst, a, b_, w):
                # alternate DVE / gpsimd for the elementwise bf16 adds
                eng_ctr[0] += 1
                if eng_ctr[0] % 3 == 0:
                    nc.gpsimd.tensor_add(dst[:, :w], a[:, :w], b_[:, :w])
                else:
                    nc.vector.tensor_add(dst[:, :w], a[:, :w], b_[:, :w])

            def tree_push(cur, w, i):
                lvl = 0
                while levels[lvl] is not None:
                    prev = levels[lvl]
                    levels[lvl] = None
                    nt = treep.tile([P, 1024], bf16, tag="tree",
                                    name=f"tr_{i}_{lvl}")
                    tree_add(nt, prev, cur, w)
                    cur = nt
                    lvl += 1
                levels[lvl] = cur

            def tree_fold(w, i, extra):
                acc = extra
                for lvl in range(6):
                    if levels[lvl] is None:
                        continue
                    if acc is None:
                        acc = levels[lvl]
                    else:
                        nt = treep.tile([P, 1024], bf16, tag="tree",
                                        name=f"tf_{i}_{lvl}")
                        tree_add(nt, acc, levels[lvl], w)
                        acc = nt
                    levels[lvl] = None
                return acc

            def make_tail(s, off, w, acc, i):
                """Deferred sums matmul + drains for a finished sub-block."""
                def tail():
                    sm_ps = sps.tile([P, 1024], f32, tag="s", name=f"sm_{i}")
                    # dummy allocations keep the s rotation parity (mod 3)
                    sps.tile([P, 1024], f32, tag="s", name=f"sdua_{i}")
                    sps.tile([P, 1024], f32, tag="s", name=f"sdub_{i}")
                    for (ws, ww) in _bank_windows(0, w):
                        nc.tensor.matmul(sm_ps[0:1, ws:ws + ww],
                                         lhsT=ones_b[:],
                                         rhs=acc[:, ws:ws + ww],
                                         start=True, stop=True)
                    sm_st = smp.tile([1, 1024], f32, tag="smst",
                                     name=f"smst_{i}")
                    nc.scalar.copy(out=sm_st[:, :w], in_=sm_ps[0:1, :w])
                    nc.gpsimd.dma_start(sums_d[s, off:off + w], sm_st[:, :w])
                return tail

            pend = {0: emit_qk(0)}
            if len(steps) > 1:
                pend[1] = emit_qk(1)
            pend_tail = []
            ot_ps = None
            for i, (s, off, w, chunks, gi, has_last, last_grp) in enumerate(steps):
                nkc, length = sched[s]
                q_sb, k_sb, v_sb = staged[s]
                if gi == 0:
                    ot_ps = ops.tile([P, 1024], f32, tag="o", name=f"ot_{i}")
                gw = len(chunks) * w
                pt = ptp.tile([P, 1024], bf16, tag="pt", name=f"pt_{i}")
                s_ps = pend.pop(i)
                use_dve = (gi % DVE_MOD == 1) and not has_last
                if use_dve:
                    nc.vector.tensor_scalar(
                        out=pt[:, :gw].bitcast(i16), in0=s_ps[:, :gw],
                        scalar1=SCHRAU_S1, scalar2=SCHRAU_S2,
                        op0=mult, op1=add_op)
                else:
                    nc.scalar.activation(pt[:, :gw], s_ps[:, :gw], Exp,
                                         scale=SCALE)
                if i + 2 < len(steps):
                    pend[i + 2] = emit_qk(i + 2)
                first_grp = gi == 0
                for j, kc in enumerate(chunks):
                    lastmm = last_grp and j == len(chunks) - 1
                    for (ws, ww) in _bank_windows(0, w):
                        nc.tensor.matmul(
                            ot_ps[:, ws:ws + ww],
                            lhsT=v_sb[:, kc, :],
                            rhs=pt[:, j * w + ws:j * w + ws + ww],
                            start=(first_grp and j == 0), stop=lastmm)
                while pend_tail and pend_tail[0][0] <= i:
                    pend_tail.pop(0)[1]()
                # fold this group's probs into the running tree
                if len(chunks) == 2:
                    pg = treep.tile([P, 1024], bf16, tag="tree",
                                    name=f"pg_{i}")
                    tree_add(pg, pt[:, 0:w], pt[:, w:2 * w], w)
                    cur = pg
                else:
                    cur = pt
                if not last_grp:
                    tree_push(cur, w, i)
                    continue

                # ---- sub-block tail ----
                acc = tree_fold(w, i, None)
                if acc is None:
                    acc = cur
                else:
                    nt = treep.tile([P, 1024], bf16, tag="tree",
                                    name=f"acF_{i}")
                    tree_add(nt, acc, cur, w)
                    acc = nt
                # drain O'^T (split per bank-window across ScalarE / DVE),
                # one DMA for the whole sub-block
                o_st = otp.tile([P, 1024], f32, tag="ost", name=f"ost_{i}")
                for wi, (ws, ww) in enumerate(_bank_windows(0, w)):
                    if wi % 2 == 0:
                        nc.vector.tensor_copy(out=o_st[:, ws:ws + ww],
                                              in_=ot_ps[:, ws:ws + ww])
                    else:
                        nc.scalar.copy(out=o_st[:, ws:ws + ww],
                                       in_=ot_ps[:, ws:ws + ww])
                nc.gpsimd.dma_start(out_d[s, :, off:off + w], o_st[:, :w])
                pend_tail.append((i + 2, make_tail(s, off, w, acc, i)))
            while pend_tail:
                pend_tail.pop(0)[1]()

    nc.compile()
    return nc


_PROG_CACHE: dict = {}


def _get_program(sched) -> bacc.Bacc:
    if sched not in _PROG_CACHE:
        _PROG_CACHE[sched] = build_program(sched)
    return _PROG_CACHE[sched]


def _plan(attn_mask):
    mf = (np.asarray(attn_mask) > 0)
    any_valid = mf.any(axis=1)
    last_plus1 = np.where(any_valid, S - np.argmax(mf[:, ::-1], axis=1), 1)
    nkc = np.maximum(1, (last_plus1 + P - 1) // P).astype(int)   # [B]
    lens = np.maximum(1, last_plus1).astype(int)
    # slot s -> batch s//2 (two head-slots per batch per core)
    sched = tuple((int(nkc[s // 2]), int(lens[s // 2])) for s in range(8))
    return mf, nkc, lens, sched


def make_in_maps(query, key, value, attn_mask):
    mf, nkc, lens, sched = _plan(attn_mask)
    # device wants q/k as [slot, D, S] (pre-transposed), v as [slot, S, D]
    qT = np.asarray(query, np.float32).transpose(0, 2, 3, 1)     # [B, H, D, S]
    kT = np.asarray(key, np.float32).transpose(0, 2, 3, 1)       # [B, H, D, S]
    v = np.asarray(value, np.float32).transpose(0, 2, 1, 3)      # [B, H, S, D]
    mff = mf.astype(np.float32)
    kTz = kT * mff[:, None, None, :]
    vz = (v * mff[:, None, :, None]).astype(ml_dtypes.bfloat16)
    in_maps = []
    for c in range(N_CORES):
        qs = np.empty((8, P, S), ml_dtypes.bfloat16)
        ks = np.empty((8, P, S), ml_dtypes.bfloat16)
        vs = np.empty((8, S, P), ml_dtypes.bfloat16)
        for s in range(8):
            b, h = s // 2, 2 * c + (s % 2)
            ln, kw = int(lens[b]), int(nkc[b]) * P
            qs[s, :, :ln] = qT[b, h, :, :ln]
            ks[s, :, :kw] = kTz[b, h, :, :kw]
            vs[s, :kw, :] = vz[b, h, :kw, :]
        in_maps.append({"q": qs, "k": ks, "v": vs})
    return in_maps, mf


def assemble_output(results, mf):
    _, nkc, lens, _ = _plan(mf.astype(np.int32))
    mcount = np.array([nkc[b] * P - mf[b, :nkc[b] * P].sum() for b in range(B)],
                      np.float32)
    out = np.zeros((B, S, H * D), np.float32)
    for c in range(N_CORES):
        for s in range(8):
            b, h = s // 2, 2 * c + (s % 2)
            ln = int(lens[b])
            oT = results[c]["out"][s][:, :ln]                    # [D, ln]
            sums = results[c]["sums_out"][s][:ln] - mcount[b]    # [ln]
            with np.errstate(divide="ignore", invalid="ignore"):
                scale = np.where(mf[b, :ln], 1.0 / sums, 0.0)
            out[b, :ln, h * D:(h + 1) * D] = (oT * scale[None, :]).T
    return out


def kernel(query, key, value, attn_mask):
    _, _, _, sched = _plan(attn_mask)
    nc = _get_program(sched)
    in_maps, mf = make_in_maps(query, key, value, attn_mask)
    res = run_bass_kernel_spmd(nc, in_maps, list(range(N_CORES)))
    return assemble_output(res.results, mf)


# revision 3
# speedup vs baseline: 1.2719x; 1.2719x over previous
"""Trainium2 Bass kernel: dense attention with key-padding mask (ColoAttention).

Math (per batch b, head h):
    scores = (Q @ K^T) / sqrt(D); masked keys -> -inf; softmax over keys;
    out = probs @ V; rows at masked query positions zeroed.

v3 design notes:
  - Sharding: every core gets 2 heads from EVERY batch (16 heads / 8 cores),
    so all cores run the identical schedule.  K/V rows at masked keys are
    zeroed on the host; exp(0)=1 contributions are subtracted from the row
    sums on the host (mcount).  Device emits unnormalized O'^T and raw sums;
    host applies qmask/(sums-mcount) and the final [D,S]->[S,D] transpose.
  - Scores are computed transposed (S^T[k,q] = K @ Q^T) so the exp output
    P^T (bf16) directly feeds O'^T = V^T @ P^T.
  - Uniform 512-wide query sub-blocks (len-exact, no padding).  Each PSUM
    score tile [128,1024]f32 holds TWO key-chunks side by side, so one
    activation covers both (halves the per-ACTIVATE fixed cost).
  - exp runs on TWO engines: ScalarE exact LUT exp for ~2/3 of chunk-pairs,
    DVE Schraudolph exp2 for the rest: one stock tensor_scalar computes
    bits16 = rint(score*128*SCALE*log2e + 128*(127+c)) written as int16
    into the bf16 prob tile -- those 16 bits ARE the bf16 pattern of
    ~2^y (rms log err ~1.8% on ~1/3 of keys -> ~1% output error vs the
    2e-2 gate).  Groups containing the last (masked-key) chunk stay on
    ScalarE so masked keys contribute exactly exp(0)=1.
  - Row sums: prob chunks tree-accumulate pairwise on the DVE (bf16 2x),
    then one ones-vector matmul per sub-block reduces the 128 partitions
    exactly in PSUM (deferred 2 groups so the PE never waits on the DVE).
    GpSimd only issues output DMAs (its ALU is ~4x slower than DVE and
    poisons the pipeline).
  - PSUM: scores [128,1024]f32 x3 rotation (6 banks; sums matmuls ride the
    rotation with dummy allocations keeping parity) + O' accum [128,512]f32
    x2 double-buffered (2 banks) = 8 banks.
"""

import numpy as np
import ml_dtypes
from contextlib import ExitStack

import concourse.bass as bass
import concourse.mybir as mybir
import concourse.tile as tile
from concourse import bacc
from concourse.bass_utils import run_bass_kernel_spmd

B, S, H, D = 4, 2048, 16, 128
N_CORES = 8
P = 128
SCALE = 1.0 / float(np.sqrt(np.float64(D)).astype(np.float32))
LOG2E = float(np.log2(np.e))
SCHRAU_C = -0.057
SCHRAU_S1 = 128.0 * SCALE * LOG2E
SCHRAU_S2 = 128.0 * (127.0 + SCHRAU_C)
DVE_MOD = 3  # group -> DVE exp when gi % DVE_MOD == 1 (and no last chunk)


def _subs_of(length: int):
    """Uniform 512-wide query sub-blocks (len-exact)."""
    return [(o, min(512, length - o)) for o in range(0, length, 512)]


def _bank_windows(lo: int, hi: int):
    """Split [lo, hi) at 512-col PSUM bank boundaries."""
    res = []
    x = lo
    while x < hi:
        nxt = min(hi, (x // 512 + 1) * 512)
        res.append((x, nxt - x))
        x = nxt
    return res


def build_program(sched) -> bacc.Bacc:
    """sched: tuple of (nkc, length) per slot, identical on every core."""
    f32 = mybir.dt.float32
    bf16 = mybir.dt.bfloat16
    i16 = mybir.dt.int16
    Exp = mybir.ActivationFunctionType.Exp
    mult = mybir.AluOpType.mult
    add_op = mybir.AluOpType.add

    nc = bacc.Bacc("TRN2", target_bir_lowering=False, debug=False)
    q_d = nc.dram_tensor("q", [8, P, S], bf16, kind="ExternalInput").ap()
    k_d = nc.dram_tensor("k", [8, P, S], bf16, kind="ExternalInput").ap()
    v_d = nc.dram_tensor("v", [8, S, P], bf16, kind="ExternalInput").ap()
    out_d = nc.dram_tensor("out", [8, P, S], f32, kind="ExternalOutput").ap()
    sums_d = nc.dram_tensor("sums_out", [8, S], f32, kind="ExternalOutput").ap()

    # steps: one entry per chunk-pair group:
    # (slot, sub_off, sub_w, chunks, gi, has_last_chunk, is_last_group)
    steps = []
    for s, (nkc, length) in enumerate(sched):
        for (off, w) in _subs_of(length):
            gs = [tuple(range(k, min(k + 2, nkc))) for k in range(0, nkc, 2)]
            for gi, chunks in enumerate(gs):
                steps.append((s, off, w, chunks, gi,
                              (nkc - 1) in chunks, gi == len(gs) - 1))

    with tile.TileContext(nc) as tc:
        with ExitStack() as ctx:
            consts = ctx.enter_context(tc.tile_pool(name="consts", bufs=1))
            qkp = ctx.enter_context(tc.tile_pool(name="qkp", bufs=2))
            ptp = ctx.enter_context(tc.tile_pool(name="ptp", bufs=4))
            treep = ctx.enter_context(tc.tile_pool(name="treep", bufs=8))
            otp = ctx.enter_context(tc.tile_pool(name="otp", bufs=4))
            smp = ctx.enter_context(tc.tile_pool(name="smp", bufs=2))
            sps = ctx.enter_context(tc.tile_pool(name="sps", bufs=3, space="PSUM"))
            ops = ctx.enter_context(tc.tile_pool(name="ops", bufs=2, space="PSUM"))

            ones_b = consts.tile([P, 1], bf16, tag="ones")
            nc.gpsimd.memset(ones_b[:], 1.0)

            staged = {}

            def stage(s):
                if s in staged or s >= len(sched):
                    return
                nkc, length = sched[s]
                kw = nkc * P
                q_sb = qkp.tile([P, length], bf16, tag="q", name=f"q_{s}")
                k_sb = qkp.tile([P, kw], bf16, tag="k", name=f"k_{s}")
                v_sb = qkp.tile([P, nkc, P], bf16, tag="v", name=f"v_{s}")
                nc.sync.dma_start(k_sb[:, 0:P], k_d[s, :, 0:P])
                nc.sync.dma_start(q_sb[:, 0:512], q_d[s, :, 0:512])
                nc.sync.dma_start(v_sb[:, 0, :], v_d[s, 0:P, :])
                if length > 512:
                    nc.sync.dma_start(q_sb[:, 512:length], q_d[s, :, 512:length])
                if nkc > 1:
                    nc.sync.dma_start(k_sb[:, P:kw], k_d[s, :, P:kw])
                    nc.sync.dma_start(
                        v_sb[:, 1:nkc, :],
                        v_d[s, P:kw, :].rearrange("(t r) d -> r t d", r=P))
                staged[s] = (q_sb, k_sb, v_sb)

            stage(0)
            stage(1)

            def emit_qk(i):
                s, off, w, chunks, gi, _, _ = steps[i]
                stage(s + 1)
                q_sb, k_sb, _ = staged[s]
                s_ps = sps.tile([P, 1024], f32, tag="s", name=f"s_{i}")
                for j, kc in enumerate(chunks):
                    for (ws, ww) in _bank_windows(j * w, (j + 1) * w):
                        qs = off + ws - j * w
                        nc.tensor.matmul(
                            s_ps[:, ws:ws + ww],
                            lhsT=k_sb[:, kc * P:(kc + 1) * P],
                            rhs=q_sb[:, qs:qs + ww],
                            start=True, stop=True)
                return s_ps

            # binary-counter tree accumulation of prob chunks (per sub-block)
            levels = [None] * 6

            def tree_push(cur, w, i):
                lvl = 0
                while levels[lvl] is not None:
                    prev = levels[lvl]
                    levels[lvl] = None
                    nt = treep.tile([P, 512], bf16, tag="tree",
                                    name=f"tr_{i}_{lvl}")
                    nc.vector.tensor_add(nt[:, :w], prev[:, :w], cur[:, :w])
                    cur = nt
                    lvl += 1
                levels[lvl] = cur

            def tree_fold(w, i):
                acc = None
                for lvl in range(6):
                    if levels[lvl] is None:
                        continue
                    if acc is None:
                        acc = levels[lvl]
                    else:
                        nt = treep.tile([P, 512], bf16, tag="tree",
                                        name=f"tf_{i}_{lvl}")
                        nc.vector.tensor_add(nt[:, :w], acc[:, :w],
                                             levels[lvl][:, :w])
                        acc = nt
                    levels[lvl] = None
                return acc

            sub_ctr = [0]

            def make_tail(s, off, w, acc, i):
                """Deferred sums matmul + drain for a finished sub-block."""
                def tail():
                    sm_ps = sps.tile([P, 1024], f32, tag="s", name=f"sm_{i}")
                    # dummy allocations keep the s rotation parity (mod 3)
                    sps.tile([P, 1024], f32, tag="s", name=f"sdua_{i}")
                    sps.tile([P, 1024], f32, tag="s", name=f"sdub_{i}")
                    nc.tensor.matmul(sm_ps[0:1, 0:w], lhsT=ones_b[:],
                                     rhs=acc[:, :w], start=True, stop=True)
                    sm_st = smp.tile([1, 512], f32, tag="smst",
                                     name=f"smst_{i}")
                    nc.scalar.copy(out=sm_st[:, :w], in_=sm_ps[0:1, :w])
                    nc.gpsimd.dma_start(sums_d[s, off:off + w], sm_st[:, :w])
                return tail

            pend = {0: emit_qk(0)}
            if len(steps) > 1:
                pend[1] = emit_qk(1)
            pend_tail = []
            ot_ps = None
            for i, (s, off, w, chunks, gi, has_last, last_grp) in enumerate(steps):
                nkc, length = sched[s]
                q_sb, k_sb, v_sb = staged[s]
                if gi == 0:
                    ot_ps = ops.tile([P, 512], f32, tag="o", name=f"ot_{i}")
                gw = len(chunks) * w
                pt = ptp.tile([P, 1024], bf16, tag="pt", name=f"pt_{i}")
                s_ps = pend.pop(i)
                use_dve = (gi % DVE_MOD == 1) and not has_last
                if use_dve:
                    nc.vector.tensor_scalar(
                        out=pt[:, :gw].bitcast(i16), in0=s_ps[:, :gw],
                        scalar1=SCHRAU_S1, scalar2=SCHRAU_S2,
                        op0=mult, op1=add_op)
                else:
                    nc.scalar.activation(pt[:, :gw], s_ps[:, :gw], Exp,
                                         scale=SCALE)
                if i + 2 < len(steps):
                    pend[i + 2] = emit_qk(i + 2)
                first_grp = gi == 0
                for j, kc in enumerate(chunks):
                    lastmm = last_grp and j == len(chunks) - 1
                    nc.tensor.matmul(
                        ot_ps[:, 0:w],
                        lhsT=v_sb[:, kc, :],
                        rhs=pt[:, j * w:j * w + w],
                        start=(first_grp and j == 0), stop=lastmm)
                while pend_tail and pend_tail[0][0] <= i:
                    pend_tail.pop(0)[1]()
                # fold this group's probs into the running tree
                if len(chunks) == 2:
                    pg = treep.tile([P, 512], bf16, tag="tree",
                                    name=f"pg_{i}")
                    nc.vector.tensor_add(pg[:, :w], pt[:, 0:w], pt[:, w:2 * w])
                    cur = pg
                else:
                    cur = pt
                if not last_grp:
                    tree_push(cur, w, i)
                    continue

                # ---- sub-block tail ----
                acc = tree_fold(w, i)
                if acc is None:
                    acc = cur
                else:
                    nt = treep.tile([P, 512], bf16, tag="tree",
                                    name=f"acF_{i}")
                    nc.vector.tensor_add(nt[:, :w], acc[:, :w], cur[:, :w])
                    acc = nt
                # drain O'^T: one copy (alternate ScalarE / DVE), one DMA
                o_st = otp.tile([P, 512], f32, tag="ost", name=f"ost_{i}")
                sub_ctr[0] += 1
                if sub_ctr[0] % 2 == 0:
                    nc.vector.tensor_copy(out=o_st[:, :w], in_=ot_ps[:, :w])
                else:
                    nc.scalar.copy(out=o_st[:, :w], in_=ot_ps[:, :w])
                nc.gpsimd.dma_start(out_d[s, :, off:off + w], o_st[:, :w])
                pend_tail.append((i + 2, make_tail(s, off, w, acc, i)))
            while pend_tail:
                pend_tail.pop(0)[1]()

    nc.compile()
    return nc


_PROG_CACHE: dict = {}


def _get_program(sched) -> bacc.Bacc:
    if sched not in _PROG_CACHE:
        _PROG_CACHE[sched] = build_program(sched)
    return _PROG_CACHE[sched]


def _plan(attn_mask):
    mf = (np.asarray(attn_mask) > 0)
    any_valid = mf.any(axis=1)
    last_plus1 = np.where(any_valid, S - np.argmax(mf[:, ::-1], axis=1), 1)
    nkc = np.maximum(1, (last_plus1 + P - 1) // P).astype(int)   # [B]
    lens = np.maximum(1, last_plus1).astype(int)
    # slot s -> batch s//2 (two head-slots per batch per core)
    sched = tuple((int(nkc[s // 2]), int(lens[s // 2])) for s in range(8))
    return mf, nkc, lens, sched


def make_in_maps(query, key, value, attn_mask):
    mf, nkc, lens, sched = _plan(attn_mask)
    # device wants q/k as [slot, D, S] (pre-transposed), v as [slot, S, D]
    qT = np.asarray(query, np.float32).transpose(0, 2, 3, 1)     # [B, H, D, S]
    kT = np.asarray(key, np.float32).transpose(0, 2, 3, 1)       # [B, H, D, S]
    v = np.asarray(value, np.float32).transpose(0, 2, 1, 3)      # [B, H, S, D]
    mff = mf.astype(np.float32)
    kTz = kT * mff[:, None, None, :]
    vz = (v * mff[:, None, :, None]).astype(ml_dtypes.bfloat16)
    in_maps = []
    for c in range(N_CORES):
        qs = np.empty((8, P, S), ml_dtypes.bfloat16)
        ks = np.empty((8, P, S), ml_dtypes.bfloat16)
        vs = np.empty((8, S, P), ml_dtypes.bfloat16)
        for s in range(8):
            b, h = s // 2, 2 * c + (s % 2)
            ln, kw = int(lens[b]), int(nkc[b]) * P
            qs[s, :, :ln] = qT[b, h, :, :ln]
            ks[s, :, :kw] = kTz[b, h, :, :kw]
            vs[s, :kw, :] = vz[b, h, :kw, :]
        in_maps.append({"q": qs, "k": ks, "v": vs})
    return in_maps, mf


def assemble_output(results, mf):
    _, nkc, lens, _ = _plan(mf.astype(np.int32))
    mcount = np.array([nkc[b] * P - mf[b, :nkc[b] * P].sum() for b in range(B)],
                      np.float32)
    out = np.zeros((B, S, H * D), np.float32)
    for c in range(N_CORES):
        for s in range(8):
            b, h = s // 2, 2 * c + (s % 2)
            ln = int(lens[b])
            oT = results[c]["out"][s][:, :ln]                    # [D, ln]
            sums = results[c]["sums_out"][s][:ln] - mcount[b]    # [ln]
            with np.errstate(divide="ignore", invalid="ignore"):
                scale = np.where(mf[b, :ln], 1.0 / sums, 0.0)
            out[b, :ln, h * D:(h + 1) * D] = (oT * scale[None, :]).T
    return out


def kernel(query, key, value, attn_mask):
    _, _, _, sched = _plan(attn_mask)
    nc = _get_program(sched)
    in_maps, mf = make_in_maps(query, key, value, attn_mask)
    res = run_bass_kernel_spmd(nc, in_maps, list(range(N_CORES)))
    return assemble_output(res.results, mf)
